# revision 19
# baseline (speedup 1.0000x reference)
"""Trainium2 Bass kernel for nn_CLA_19636590478164 (sparse deformable attention).

Strategy (8 cores, SPMD):
  core i handles batch b = i//2 and ref-pair j = i%2 (refs 2j, 2j+1 -> k = 8j..8j+7).
  On-chip per core:
    1. QO matmul: per-pixel offset (9 ch) + attn (16 ch) logits via PE.
    2. Softmax (ACT exp) + bilinear weights + gather indices on DVE.
    3. kf = w_ref @ key per ref via PE -> fp16 -> zero-padded 66x66 grid in DRAM.
    4. Per (k, pixel): 2 indirect-DMA gathers (row y0 and y0+1, 2 pixels x 256ch
       fp16 each); combine = sum_k,corner diag(w) @ g via PE accumulating in PSUM.
  Host: sum the two ref-pair partials per batch, transpose to channels-major.
"""

import numpy as np

import concourse.bass as bass
import concourse.mybir as mybir
import concourse.tile as tile
from concourse.masks import make_identity

F16 = mybir.dt.float16
F32 = mybir.dt.float32
I32 = mybir.dt.int32
I16 = mybir.dt.int16
AO = mybir.AluOpType

B, C, H, W = 4, 256, 64, 64
HW = H * W            # 4096
NK = 8                # k's per core
NCHUNK = 32           # 128-pixel chunks
G = 66                # padded grid side
RG = G * G            # 4356 grid cells
NQO = 25              # 9 off + 16 attn columns


def build_body(tc, outp, ins, dbg=None):
    """outp: [4096, 256] f32 DRAM AP; ins: dict of DRAM APs."""
    nc = tc.nc
    q_d, k_d = ins["q16"], ins["k16"]
    wqo_d, bqo_d = ins["wqo"], ins["bqo"]
    wref_d, bref_d = ins["wref"], ins["bref"]

    with (
        tc.tile_pool(name="const", bufs=1) as cp,
        tc.tile_pool(name="tmp", bufs=2) as tp,
        tc.tile_pool(name="kin", bufs=2) as kp,
        tc.tile_pool(name="kf", bufs=1) as kfp,
        tc.tile_pool(name="ga", bufs=2) as gap,
        tc.tile_pool(name="gb", bufs=2) as gbp,
        tc.tile_pool(name="diag", bufs=8) as dgp,
        tc.tile_pool(name="osb", bufs=1) as op_,
        tc.tile_pool(name="acc", bufs=8, space="PSUM") as pac,
        tc.tile_pool(name="dram", bufs=1, space="DRAM") as dp,
    ):
        # ---- constants / inputs resident in SBUF ----
        q_sb = kp.tile([128, 2 * HW], F16, tag="qk", name="q_sb")
        nc.sync.dma_start(out=q_sb[:].rearrange("p (a x) -> p a x", a=2), in_=q_d.rearrange("(a p) x -> p a x", p=128))
        wqo_sb = cp.tile([128, 2 * NQO], F16, tag="wqo")
        nc.sync.dma_start(out=wqo_sb[:].rearrange("p (a n) -> p a n", a=2), in_=wqo_d.rearrange("(a p) n -> p a n", p=128))
        wref_sb = cp.tile([128, 2 * C], F16, tag="wref")
        nc.sync.dma_start(out=wref_sb[:].rearrange("p (a n) -> p a n", a=2), in_=wref_d.rearrange("(a p) n -> p a n", p=128))
        bqo_sb = cp.tile([1, NQO], F16, tag="bqo")
        nc.sync.dma_start(out=bqo_sb[:], in_=bqo_d)
        bref_sb = cp.tile([1, C], F16, tag="bref")
        nc.sync.dma_start(out=bref_sb[:], in_=bref_d)

        ones = cp.tile([1, 128], F16, tag="ones")
        nc.vector.memset(ones[:], 1.0)
        ident = cp.tile([128, 128], F16, tag="ident")
        make_identity(nc, ident[:])
        zero_sb = cp.tile([128, 512], F16, tag="zero")
        nc.vector.memset(zero_sb[:], 0.0)

        # pixel coordinate grids: p = chunk*128 + part; y = 2*chunk + part//64, x = part%64
        pidx_i = cp.tile([128, 1], I32, tag="pidx_i")
        nc.gpsimd.iota(pidx_i[:], pattern=[[0, 1]], base=0, channel_multiplier=1)
        pidx_f = cp.tile([128, 1], F32, tag="pidx_f")
        nc.vector.tensor_copy(out=pidx_f[:], in_=pidx_i[:])
        ge64 = cp.tile([128, 1], F32, tag="ge64")
        nc.vector.tensor_scalar(out=ge64[:], in0=pidx_f[:], scalar1=64.0, scalar2=None, op0=AO.is_ge)
        xgrid = cp.tile([128, 1], F32, tag="xgrid")  # p % 64
        nc.vector.scalar_tensor_tensor(out=xgrid[:], in0=ge64[:], scalar=-64.0, in1=pidx_f[:], op0=AO.mult, op1=AO.add)
        yrow_i = cp.tile([128, NCHUNK], I32, tag="yrow_i")
        nc.gpsimd.iota(yrow_i[:], pattern=[[2, NCHUNK]], base=0, channel_multiplier=0)
        ygrid = cp.tile([128, NCHUNK], F32, tag="ygrid")  # 2*chunk + part//64
        nc.vector.tensor_tensor(out=ygrid[:], in0=yrow_i[:], in1=ge64[:].to_broadcast([128, NCHUNK]), op=AO.add)

        # ---- phase 1: QO logits [128, 32*25] f32 ----
        qo = cp.tile([128, NCHUNK * NQO], F32, tag="qo")
        for ch in range(NCHUNK):
            ps = pac.tile([128, NQO], F32, tag="acc", name=f"psq{ch}")
            for a in range(2):
                nc.tensor.matmul(
                    out=ps[:], lhsT=q_sb[:, a * HW + ch * 128:a * HW + ch * 128 + 128],
                    rhs=wqo_sb[:, a * NQO:(a + 1) * NQO],
                    start=(a == 0), stop=False, skip_group_check=True)
            nc.tensor.matmul(out=ps[:], lhsT=ones[:, :], rhs=bqo_sb[:, :],
                             start=False, stop=True, skip_group_check=True)
            nc.scalar.copy(out=qo[:, ch * NQO:(ch + 1) * NQO], in_=ps[:])

        def qo_view(col0, nk=NK, kstep=1):
            # [128, nk, 32] view of qo: element (p, kl, ch) = qo[p, ch*25 + col0 + kl*kstep]
            return bass.AP(qo[:].tensor, qo[:].offset + col0,
                           [qo[:].ap[0], [kstep, nk], [NQO, NCHUNK]])

        # ---- softmax over the 16 attn logits (cols 9..24) ----
        logit = qo_view(9, nk=16)                       # [128, 16, 32] (kstep=1 -> k inner? no: [kstep,nk]=[1,16] middle, chunk outer stride 25)
        # reduce innermost: want max over the 16 k's per chunk -> put k innermost:
        logit_ki = bass.AP(qo[:].tensor, qo[:].offset + 9, [qo[:].ap[0], [NQO, NCHUNK], [1, 16]])
        mx = tp.tile([128, NCHUNK], F32, tag="mx")
        nc.vector.tensor_reduce(out=mx[:], in_=logit_ki, axis=mybir.AxisListType.X, op=AO.max)
        ew = cp.tile([128, NCHUNK * 16], F32, tag="ew")
        mx_b = bass.AP(mx[:].tensor, mx[:].offset, [mx[:].ap[0], [1, NCHUNK], [0, 16]])
        nc.vector.tensor_tensor(out=ew[:], in0=logit_ki, in1=mx_b, op=AO.subtract)
        nc.scalar.activation(out=ew[:], in_=ew[:], func=mybir.ActivationFunctionType.Exp)
        sm = tp.tile([128, NCHUNK], F32, tag="sm")
        ew3 = bass.AP(ew[:].tensor, ew[:].offset, [ew[:].ap[0], [16, NCHUNK], [1, 16]])
        nc.vector.tensor_reduce(out=sm[:], in_=ew3, axis=mybir.AxisListType.X, op=AO.add)
        rs = tp.tile([128, NCHUNK], F32, tag="rs")
        nc.vector.reciprocal(out=rs[:], in_=sm[:])
        attn = cp.tile([128, NCHUNK * 16], F32, tag="attn")  # [128, ch, 16] dense
        rs_b = bass.AP(rs[:].tensor, rs[:].offset, [rs[:].ap[0], [1, NCHUNK], [0, 16]])
        nc.vector.tensor_tensor(out=attn[:], in0=ew3, in1=rs_b, op=AO.mult)

        # ---- phase 2: bilinear weights + gather indices ([128, 8, 32] dense) ----
        def kwide(tag):
            return cp.tile([128, NK * NCHUNK], F32, tag=tag, name=tag)

        def ap3(t):  # [128, 8, 32] view of dense [128, 256]
            return t[:].rearrange("p (k c) -> p k c", k=NK)

        py, px_ = kwide("py"), kwide("px")
        yg_b = bass.AP(ygrid[:].tensor, ygrid[:].offset, [ygrid[:].ap[0], [0, NK], [1, NCHUNK]])
        xg_b = bass.AP(xgrid[:].tensor, xgrid[:].offset, [xgrid[:].ap[0], [0, NK], [0, NCHUNK]])
        nc.vector.tensor_tensor(out=ap3(py), in0=qo_view(0), in1=yg_b, op=AO.add)
        nc.vector.tensor_tensor(out=ap3(px_), in0=qo_view(1), in1=xg_b, op=AO.add)

        def floor_split(v, ltag, ftag):
            # floor, robust to the f32->i32 cast rounding mode (sim truncates,
            # HW rounds to nearest): candidate = castback(cast(v + 1024)),
            # then subtract 1 wherever candidate > v.
            t = kwide(ltag + "_t")
            nc.vector.tensor_scalar(out=t[:], in0=v[:], scalar1=1024.0, scalar2=None, op0=AO.add)
            ti = cp.tile([128, NK * NCHUNK], I32, tag=ltag + "_i", name=ltag + "_i")
            nc.vector.tensor_copy(out=ti[:], in_=t[:])
            tf = kwide(ltag + "_f")
            nc.vector.tensor_copy(out=tf[:], in_=ti[:])
            d = kwide(ltag + "_d")
            nc.vector.tensor_tensor(out=d[:], in0=t[:], in1=tf[:], op=AO.is_lt)
            ysh = kwide(ltag + "_s")
            nc.vector.tensor_tensor(out=ysh[:], in0=tf[:], in1=d[:], op=AO.subtract)
            lf = kwide(ltag)
            nc.vector.tensor_tensor(out=lf[:], in0=t[:], in1=ysh[:], op=AO.subtract)
            fl = kwide(ftag)
            nc.vector.tensor_scalar(out=fl[:], in0=ysh[:], scalar1=-1024.0, scalar2=None, op0=AO.add)
            return lf, fl

        ly, y0 = floor_split(py, "ly", "y0")
        lx, x0 = floor_split(px_, "lx", "x0")

        def clipped(src, lo, hi, tag):
            o = kwide(tag)
            nc.vector.tensor_scalar(out=o[:], in0=src[:], scalar1=hi, scalar2=lo, op0=AO.min, op1=AO.max)
            return o

        gy = clipped(y0, -1.0, 63.0, "gy")
        gx = clipped(x0, -1.0, 63.0, "gx")

        def validity(src, lo, hi, tag):
            cl = clipped(src, lo, hi, tag + "c")
            v = kwide(tag)
            nc.vector.tensor_tensor(out=v[:], in0=cl[:], in1=src[:], op=AO.is_equal)
            return v

        vy0 = validity(y0, 0.0, 63.0, "vy0")
        vy1 = validity(y0, -1.0, 62.0, "vy1")
        vx0 = validity(x0, 0.0, 63.0, "vx0")
        vx1 = validity(x0, -1.0, 62.0, "vx1")

        idxf = kwide("idxf")
        nc.vector.scalar_tensor_tensor(out=idxf[:], in0=gy[:], scalar=float(G), in1=gx[:], op0=AO.mult, op1=AO.add)
        idx = cp.tile([128, NK * NCHUNK], I16, tag="idx")
        nc.vector.tensor_scalar(out=idx[:], in0=idxf[:], scalar1=float(G + 1), scalar2=None, op0=AO.add)
        # fold idx [128 p, (k ch)] -> wrapped [16 pp, (k ch q)] via DRAM roundtrip
        # (p = q*16 + pp; dma_gather consumes indices as [16, n/16] with
        #  list position j = s*16 + pp, s = ch*8 + q  =>  j = pixel index)
        didx = dp.tile([NK * NCHUNK * 128], I16, tag="didx", name="didx")
        nc.sync.dma_start(out=didx[:].rearrange("(p c) -> p c", p=128), in_=idx[:])
        idx16 = cp.tile([128, NK * NCHUNK * 8], I16, tag="idx16")
        nc.sync.dma_start(
            out=idx16[:16, :].rearrange("p (k c q) -> p k c q", k=NK, q=8),
            in_=bass.AP(didx[:].tensor, didx[:].offset,
                        [[NK * NCHUNK, 16], [NCHUNK, NK], [1, NCHUNK], [NK * NCHUNK * 16, 8]]))
        for rr in range(1, 8):
            nc.sync.dma_start(out=idx16[16 * rr:16 * rr + 16, :], in_=idx16[:16, :])

        oly = kwide("oly")
        nc.vector.tensor_scalar(out=oly[:], in0=ly[:], scalar1=-1.0, scalar2=1.0, op0=AO.mult, op1=AO.add)
        olx = kwide("olx")
        nc.vector.tensor_scalar(out=olx[:], in0=lx[:], scalar1=-1.0, scalar2=1.0, op0=AO.mult, op1=AO.add)

        attn_k = bass.AP(attn[:].tensor, attn[:].offset, [attn[:].ap[0], [1, NK], [16, NCHUNK]])
        wy0, wy1, wx0, wx1 = kwide("wy0"), kwide("wy1"), kwide("wx0"), kwide("wx1")
        nc.vector.tensor_tensor(out=wy0[:], in0=oly[:], in1=vy0[:], op=AO.mult)
        nc.vector.tensor_tensor(out=ap3(wy0), in0=ap3(wy0), in1=attn_k, op=AO.mult)
        nc.vector.tensor_tensor(out=wy1[:], in0=ly[:], in1=vy1[:], op=AO.mult)
        nc.vector.tensor_tensor(out=ap3(wy1), in0=ap3(wy1), in1=attn_k, op=AO.mult)
        nc.vector.tensor_tensor(out=wx0[:], in0=olx[:], in1=vx0[:], op=AO.mult)
        nc.vector.tensor_tensor(out=wx1[:], in0=lx[:], in1=vx1[:], op=AO.mult)

        wc = {}
        for cn, a, b_ in (("00", wy0, wx0), ("01", wy0, wx1), ("10", wy1, wx0), ("11", wy1, wx1)):
            wc[cn] = kwide("w" + cn)
            nc.vector.tensor_tensor(out=wc[cn][:], in0=a[:], in1=b_[:], op=AO.mult)

        if dbg is not None:
            for nm, t in (("qo", qo), ("attn", attn), ("idx", idx), ("w00", wc["00"]),
                          ("w01", wc["01"]), ("w10", wc["10"]), ("w11", wc["11"])):
                if nm in dbg:
                    nc.sync.dma_start(out=dbg[nm], in_=t[:])

        # ---- phase 3: kf grids (per ref) ----
        grids = []
        for r in range(2):
            grid = dp.tile([RG, C], F16, tag=f"grid{r}")
            grids.append(grid)
            k_sb = kp.tile([128, 2 * HW], F16, tag="qk", name=f"k_sb{r}")
            nc.sync.dma_start(
                out=k_sb[:].rearrange("p (a x) -> p a x", a=2),
                in_=k_d[r * C:(r + 1) * C, :].rearrange("(a p) x -> p a x", p=128))
            kf_sb = kfp.tile([128, NCHUNK * C], F16, tag="kf")
            for ch in range(NCHUNK):
                ps = pac.tile([128, C], F32, tag="acc", name=f"psk{r}_{ch}")
                for a in range(2):
                    nc.tensor.matmul(
                        out=ps[:], lhsT=k_sb[:, a * HW + ch * 128:a * HW + ch * 128 + 128],
                        rhs=wref_sb[:, a * C:(a + 1) * C],
                        start=(a == 0), stop=False, skip_group_check=True)
                nc.tensor.matmul(out=ps[:], lhsT=ones[:, :], rhs=bref_sb[:, :],
                                 start=False, stop=True, skip_group_check=True)
                nc.scalar.copy(out=kf_sb[:, ch * C:(ch + 1) * C], in_=ps[:])

            gt = grid[:, :].tensor
            # borders: top row, bottom row, left col, right col
            nc.sync.dma_start(out=bass.AP(gt, 0, [[512, 33], [1, 512]]), in_=zero_sb[:33, :512])
            nc.sync.dma_start(out=bass.AP(gt, 4290 * C, [[512, 33], [1, 512]]), in_=zero_sb[:33, :512])
            nc.sync.dma_start(out=bass.AP(gt, G * C, [[G * C, 64], [1, C]]), in_=zero_sb[:64, :C])
            nc.sync.dma_start(out=bass.AP(gt, G * C + 65 * C, [[G * C, 64], [1, C]]), in_=zero_sb[:64, :C])
            # interior: (p0, ch, c) -> cell ((2ch + p1 + 1)*66 + p0 + 1), split by p1
            for p1 in range(2):
                dst = bass.AP(gt, ((p1 + 1) * G + 1) * C,
                              [[C, 64], [2 * G * C, NCHUNK], [1, C]])
                nc.sync.dma_start(
                    out=dst,
                    in_=kf_sb[p1 * 64:(p1 + 1) * 64, :].rearrange("p (ch c) -> p ch c", c=C))

        if dbg is not None and "grid0" in dbg:
            gsb = cp.tile([128, 16 * C], F16, tag="gsb", name="gsb")
            nc.sync.dma_start(out=gsb[:].rearrange("p (f c) -> p f c", c=C), in_=grids[0][:2048, :].rearrange("(p f) c -> p f c", p=128))
            nc.sync.dma_start(out=dbg["grid0"], in_=gsb[:])

        # ---- phase 4: gather + diag-matmul combine ----
        # One PSUM bank per chunk: matmul start=True zeroes the whole 2KB bank,
        # so co-residing two accumulation chunks in one bank is unsafe.
        corner_plan = (("00", 0, 0), ("01", 0, C), ("10", 1, 0), ("11", 1, C))
        NQ = 8  # chunks per gather group
        for quarter in range(4):
            accs = [pac.tile([128, C], F32, tag="acc", name=f"acc{quarter}_{t}")
                    for t in range(NQ)]
            for kl in range(NK):
                grid = grids[kl // 4]
                gt = grid[:, :].tensor
                isl = idx16[:, kl * NCHUNK * 8 + quarter * NQ * 8:
                            kl * NCHUNK * 8 + (quarter + 1) * NQ * 8]
                nidx = NQ * 128
                gA = gap.tile([128, NQ * 512], F16, tag="ga")
                nc.gpsimd.dma_gather(
                    out_ap=gA[:].rearrange("p (m e) -> p m e", e=512),
                    in_ap=bass.AP(gt, 0, [[C, RG - 1], [1, 512]]),
                    idxs_ap=isl, num_idxs=nidx, num_idxs_reg=nidx,
                    elem_size=512, elem_step=C)
                gB = gbp.tile([128, NQ * 512], F16, tag="gb")
                nc.gpsimd.dma_gather(
                    out_ap=gB[:].rearrange("p (m e) -> p m e", e=512),
                    in_ap=bass.AP(gt, G * C, [[C, RG - G - 1], [1, 512]]),
                    idxs_ap=isl, num_idxs=nidx, num_idxs_reg=nidx,
                    elem_size=512, elem_step=C)
                if dbg is not None and quarter == 0 and kl == 0:
                    if "gA00" in dbg:
                        nc.sync.dma_start(out=dbg["gA00"], in_=gA[:])
                    if "gB00" in dbg:
                        nc.sync.dma_start(out=dbg["gB00"], in_=gB[:])
                for m in range(NQ):
                    ch = quarter * NQ + m
                    for ci, (cn, which, coff) in enumerate(corner_plan):
                        gsl = (gA if which == 0 else gB)[:, m * 512 + coff:m * 512 + coff + C]
                        dg = dgp.tile([128, 128], F16, tag="diag")
                        nc.vector.tensor_scalar_mul(
                            dg[:], ident[:], wc[cn][:, kl * NCHUNK + ch:kl * NCHUNK + ch + 1])
                        nc.tensor.matmul(
                            out=accs[m][:], lhsT=dg[:], rhs=gsl,
                            start=(kl == 0 and ci == 0), stop=(kl == NK - 1 and ci == 3),
                            skip_group_check=True)
            out_sb = op_.tile([128, NQ * C], F32, tag="osb")
            for t in range(NQ):
                nc.scalar.copy(out=out_sb[:, t * C:(t + 1) * C], in_=accs[t][:])
            nc.sync.dma_start(
                out=outp.rearrange("(ch p) c -> p ch c", p=128)[:, quarter * NQ:(quarter + 1) * NQ],
                in_=out_sb[:].rearrange("p (ch c) -> p ch c", c=C))


# ---------------------------------------------------------------------------
_CACHE = {}


def _get_nc():
    if "nc" not in _CACHE:
        from concourse import bacc
        nc = bacc.Bacc("TRN2", debug=False, num_devices=8)
        outp = nc.dram_tensor("outp", [HW, C], F32, kind="ExternalOutput")
        ins = {
            "q16": nc.dram_tensor("q16", [C, HW], F16, kind="ExternalInput"),
            "k16": nc.dram_tensor("k16", [2 * C, HW], F16, kind="ExternalInput"),
            "wqo": nc.dram_tensor("wqo", [C, NQO], F16, kind="ExternalInput"),
            "bqo": nc.dram_tensor("bqo", [1, NQO], F16, kind="ExternalInput"),
            "wref": nc.dram_tensor("wref", [C, C], F16, kind="ExternalInput"),
            "bref": nc.dram_tensor("bref", [1, C], F16, kind="ExternalInput"),
        }
        with tile.TileContext(nc) as tc:
            build_body(tc, outp.ap(), {k: v.ap() for k, v in ins.items()})
        nc.compile()
        _CACHE["nc"] = nc
    return _CACHE["nc"]


def _get_runner():
    """Cached sharded-jit runner over the 8 cores.

    Returns (run, in_names, out_names): run(concat_inputs_list) -> out_arrs.
    """
    if "runner" in _CACHE:
        return _CACHE["runner"]
    import jax
    import jax.numpy as jnp
    from jax.sharding import Mesh, PartitionSpec
    from jax.experimental.shard_map import shard_map
    import concourse.mybir as mybir_
    from concourse.bass2jax import _bass_exec_p, install_neuronx_cc_hook, partition_id_tensor

    nc = _get_nc()
    install_neuronx_cc_hook()
    in_names, out_names, out_avals, zero_shapes = [], [], [], []
    for alloc in nc.m.functions[0].allocations:
        if not isinstance(alloc, mybir_.MemoryLocationSet):
            continue
        name = alloc.memorylocations[0].name
        if alloc.kind == "ExternalInput":
            if nc.partition_id_tensor is not None and name == nc.partition_id_tensor.name:
                continue
            in_names.append(name)
        elif alloc.kind == "ExternalOutput":
            out_names.append(name)
            shape = tuple(alloc.tensor_shape)
            dtype = mybir_.dt.np(alloc.dtype)
            out_avals.append(jax.core.ShapedArray(shape, dtype))
            zero_shapes.append((shape, dtype))
    n_params = len(in_names)
    partition_name = nc.partition_id_tensor.name if nc.partition_id_tensor else None
    all_names = in_names + out_names
    if partition_name is not None:
        all_names = all_names + [partition_name]

    def _body(*args):
        operands = list(args)
        if partition_name is not None:
            operands.append(partition_id_tensor())
        outs = _bass_exec_p.bind(
            *operands,
            out_avals=tuple(out_avals),
            in_names=tuple(all_names),
            out_names=tuple(out_names),
            lowering_input_output_aliases=(),
            sim_require_finite=True,
            sim_require_nnan=True,
            nc=nc,
        )
        return tuple(outs)

    devices = jax.devices()[:8]
    mesh = Mesh(np.asarray(devices), ("core",))
    n_outs = len(out_names)
    sharded = jax.jit(
        shard_map(_body, mesh=mesh,
                  in_specs=(PartitionSpec("core"),) * (n_params + n_outs),
                  out_specs=(PartitionSpec("core"),) * n_outs,
                  check_rep=False),
        donate_argnums=tuple(range(n_params, n_params + n_outs)),
        keep_unused=True,
    )
    shardings = [jax.sharding.NamedSharding(mesh, PartitionSpec("core"))] * n_outs

    def make_zeros():
        return [
            jax.device_put(jnp.zeros((8 * s[0], *s[1:]), d), sh)
            for (s, d), sh in zip(zero_shapes, shardings)
        ]

    _CACHE["runner"] = (sharded, in_names, out_names, make_zeros, mesh)
    return _CACHE["runner"]


def make_in_maps(query_layer, key_layers, w_off, b_off, w_attn, b_attn, w_ref, b_ref):
    f16 = np.float16
    in_maps = []
    wqo_j, bqo_j = [], []
    for j in range(2):
        perm = list(range(8 * j, 8 * j + 8)) + list(range(8 * (1 - j), 8 * (1 - j) + 8))
        wqo = np.concatenate([w_off[8 * j:8 * j + 9], w_attn[perm]], axis=0)
        bqo = np.concatenate([b_off[8 * j:8 * j + 9], b_attn[perm]])
        wqo_j.append(np.ascontiguousarray(wqo.T).astype(f16))
        bqo_j.append(bqo.reshape(1, NQO).astype(f16))
    wref = np.ascontiguousarray(w_ref.T).astype(f16)
    bref = b_ref.reshape(1, C).astype(f16)
    for i in range(8):
        bi, j = i // 2, i % 2
        q16 = query_layer[bi].reshape(C, HW).astype(f16)
        k16 = key_layers[2 * j:2 * j + 2, bi].reshape(2 * C, HW).astype(f16)
        in_maps.append({
            "q16": q16, "k16": np.ascontiguousarray(k16),
            "wqo": wqo_j[j], "bqo": bqo_j[j],
            "wref": wref, "bref": bref,
        })
    return in_maps


def _concat_inputs(in_maps, in_names):
    return [np.concatenate([m[n] for m in in_maps], axis=0) for n in in_names]


def kernel(**inputs):
    sharded, in_names, out_names, make_zeros, mesh = _get_runner()
    in_maps = make_in_maps(**inputs)
    concat_in = _concat_inputs(in_maps, in_names)
    out_arrs = sharded(*concat_in, *make_zeros())
    outp = np.asarray(out_arrs[out_names.index("outp")]).reshape(8, HW, C)
    out = np.empty((B, C, H, W), np.float32)
    for bi in range(B):
        out[bi] = (outp[2 * bi] + outp[2 * bi + 1]).T.reshape(C, H, W)
    return out


def time_kernel(inputs, iters=10):
    """Return per-call seconds over a pipelined loop with device-held inputs."""
    import time
    import jax
    sharded, in_names, out_names, make_zeros, mesh = _get_runner()
    from jax.sharding import PartitionSpec
    in_maps = make_in_maps(**inputs)
    concat_in = _concat_inputs(in_maps, in_names)
    sh = jax.sharding.NamedSharding(mesh, PartitionSpec("core"))
    dev_in = [jax.device_put(a, sh) for a in concat_in]
    # warmup
    out = sharded(*dev_in, *make_zeros())
    jax.block_until_ready(out)
    zs = [make_zeros() for _ in range(iters)]
    t0 = time.time()
    outs = []
    for i in range(iters):
        outs.append(sharded(*dev_in, *zs[i]))
    jax.block_until_ready(outs)
    dt = (time.time() - t0) / iters
    return dt
